# revision 2
# baseline (speedup 1.0000x reference)
"""Trainium2 Bass kernel v2: causal MHA (B=4, T=2048, D=1024, H=16).

Sharding: batch x head-half. Core c handles batch b=c//2 and heads
[8*hh, 8*hh+8) with hh=c%2 (512 of the 1024 q/k/v dims). It computes the
partial output y_c = attn(x_b; heads hh) @ wo[:, slice]^T; the full output
is y[b] = y_{2b} + y_{2b+1} (summed on host).

All matmul data is bf16 (fp32 PSUM accumulation). Per-core dataflow:
  x_b^T resident in SBUF as [128, ko=8, T]                    (4 MB)
  v_nat[tn]  = x_chunk^T-contract @ wv      [128 tok, 512 hd] -> va
               (v in natural [token, hd] layout: no PE transposes)
  qT_g,kT_g  = w_g @ x^T                    [128, T] per chunk g (2 heads)
  S^T block  = kT_blk^T-contract qT         [tk=128, tq<=512], both heads'
               S matmuls run concurrently via PE row-tiling (base 0/64)
  E = exp(S^T * scale)  (ACT), causal mask on diagonal blocks (DVE)
  PV: po[65, tq] += [v|1]^T-contract E      (row 64 = Z)
  normalize: Z rows -> recip_approx_fast -> gpsimd partition_broadcast ->
             fused (PSUM->SBUF) multiply into oT_g
  y tile = oT_g^T-contract @ woT (accum over g) -> bf16 -> DRAM
"""

import os
import numpy as np

import concourse.bass as bass
import concourse.bacc as bacc
import concourse.mybir as mybir
from concourse.tile import TileContext
from contextlib import ExitStack

B, T, D, H = 4, 2048, 1024, 16
HD = D // H            # 64 head dim
P = 128                # partitions
KO = D // P            # 8 contraction subtiles for projections
TQT = 512              # tq tile width
NBLK = T // P          # 16 tk blocks
HPC = 8                # heads per core
G = 4                  # head-chunks per core (2 heads each)
JD = HPC * HD          # 512 local q/k/v dims
NCORES = 8
SCALE = 1.0 / float(np.sqrt(np.float32(HD)))

F32 = mybir.dt.float32
BF16 = mybir.dt.bfloat16

Exp = mybir.ActivationFunctionType.Exp
Copy = mybir.ActivationFunctionType.Copy
Mult = mybir.AluOpType.mult

# how 1/Z reaches the oT normalize: broadcast straight from PSUM (fast path)
# or staged via ACT copies into 32-aligned SBUF rows (fallback)
Z_MODE = os.environ.get("BASS_Z_MODE", "act_sbuf")
DEBUG = os.environ.get("BASS_DEBUG", "0") == "1"


def build_program():
    nc = bacc.Bacc("TRN2", target_bir_lowering=False, num_devices=NCORES)
    xt = nc.dram_tensor("xt", [D, T], BF16, kind="ExternalInput")
    wq = nc.dram_tensor("wq", [D, JD], BF16, kind="ExternalInput")
    wk = nc.dram_tensor("wk", [D, JD], BF16, kind="ExternalInput")
    wv = nc.dram_tensor("wv", [D, JD], BF16, kind="ExternalInput")
    wo = nc.dram_tensor("wo", [JD, D], BF16, kind="ExternalInput")
    cm = nc.dram_tensor("cmask", [4, P, TQT], F32, kind="ExternalInput")
    y = nc.dram_tensor("y", [T, D], BF16, kind="ExternalOutput")
    if DEBUG:
        dq = nc.dram_tensor("dq", [P, T], BF16, kind="ExternalOutput")
        dk = nc.dram_tensor("dk", [P, T], BF16, kind="ExternalOutput")
        dva = nc.dram_tensor("dva", [P, NBLK, HPC, HD + 1], BF16,
                             kind="ExternalOutput")
        det = nc.dram_tensor("det", [P, 2, TQT], BF16, kind="ExternalOutput")
        dz = nc.dram_tensor("dz", [33, TQT], F32, kind="ExternalOutput")
        do = nc.dram_tensor("do", [P, T], BF16, kind="ExternalOutput")
    dbg = {"dq": dq, "dk": dk, "dva": dva, "det": det, "dz": dz,
           "do": do} if DEBUG else None

    xt_r = xt[:].rearrange("(ko p) t -> p ko t", p=P)
    y_r = y[:].rearrange("(tn p) c -> p tn c", p=P)

    with TileContext(nc) as tc, ExitStack() as ctx:
        const = ctx.enter_context(tc.tile_pool(name="const", bufs=1))
        va_pool = ctx.enter_context(tc.tile_pool(name="va", bufs=1))
        qk_pool = ctx.enter_context(tc.tile_pool(name="qk", bufs=2))
        o_pool = ctx.enter_context(tc.tile_pool(name="o", bufs=4))
        e_pool = ctx.enter_context(tc.tile_pool(name="e", bufs=3))
        z_pool = ctx.enter_context(tc.tile_pool(name="z", bufs=2))
        y_pool = ctx.enter_context(tc.tile_pool(name="yp", bufs=2))
        psA = ctx.enter_context(tc.tile_pool(name="psA", bufs=2, space="PSUM"))
        psS = ctx.enter_context(tc.tile_pool(name="psS", bufs=2, space="PSUM"))
        psO = ctx.enter_context(tc.tile_pool(name="psO", bufs=2, space="PSUM"))

        # --- constants into SBUF ---
        x_sb = const.tile([P, KO, T], BF16, tag="x")
        for tt in range(T // TQT):
            nc.sync.dma_start(
                x_sb[:, :, tt * TQT : (tt + 1) * TQT],
                xt_r[:, :, tt * TQT : (tt + 1) * TQT],
            )
        wq_sb = const.tile([P, KO, JD], BF16, tag="wq")
        wk_sb = const.tile([P, KO, JD], BF16, tag="wk")
        wv_sb = const.tile([P, KO, JD], BF16, tag="wv")
        for w_sb, w_d in ((wq_sb, wq), (wk_sb, wk), (wv_sb, wv)):
            nc.sync.dma_start(w_sb, w_d[:].rearrange("(ko p) j -> p ko j", p=P))
        wo_sb = const.tile([P, G, D], BF16, tag="wo")
        nc.sync.dma_start(wo_sb, wo[:].rearrange("(g p) i -> p g i", p=P))
        cm_sb = const.tile([P, 4, TQT], F32, tag="cm")
        nc.sync.dma_start(cm_sb, cm[:].rearrange("m p t -> p m t"))

        # v in natural layout + shared ones column per (block, head)
        va = va_pool.tile([P, NBLK, HPC, HD + 1], BF16, tag="va")
        nc.vector.tensor_copy(
            va[:, :, :, HD : HD + 1],
            nc.const_aps.tensor(1.0, (P, NBLK, HPC, 1), F32),
        )

        def stage_v(tn):
            """v projection for token block tn, all 8 heads, natural layout."""
            psv = psA.tile([P, HPC, HD], F32, tag="psA", name=f"psv{tn}")
            for ko in range(KO):
                nc.tensor.matmul(
                    psv,
                    x_sb[:, ko, tn * P : (tn + 1) * P],
                    wv_sb[:, ko, :],
                    start=(ko == 0),
                    stop=(ko == KO - 1),
                )
            nc.vector.tensor_copy(va[:, tn, :, 0:HD], psv)

        def stage_a(g):
            """q/k projections for head-chunk g (dims 128g..128g+128)."""
            qT = qk_pool.tile([P, T], BF16, tag="qT", name=f"qT{g}")
            kT = qk_pool.tile([P, T], BF16, tag="kT", name=f"kT{g}")
            js = slice(g * P, (g + 1) * P)
            for tt in range(T // TQT):
                ts = slice(tt * TQT, (tt + 1) * TQT)
                for w_sb, dst in ((wq_sb, qT), (wk_sb, kT)):
                    pp = psA.tile([P, TQT], F32, tag="psA")
                    for ko in range(KO):
                        nc.tensor.matmul(
                            pp,
                            w_sb[:, ko, js],
                            x_sb[:, ko, ts],
                            start=(ko == 0),
                            stop=(ko == KO - 1),
                        )
                    nc.vector.tensor_copy(dst[:, ts], pp)
            return qT, kT

        def stage_b_qt(g, qT, kT, oT, qt):
            """Attention for head-chunk g, query tile qt."""
            tq0 = qt * TQT
            nblk = qt * 4 + 4
            po = [
                psO.tile([HD + 1, TQT], F32, tag="po", name=f"po{h}")
                for h in range(2)
            ]
            for kb in range(nblk):
                m = kb - qt * 4  # >=0: diagonal-crossing block
                c0 = P * m if m >= 0 else 0
                ps2 = psS.tile([P, 2, TQT], F32, tag="ps")
                for h in range(2):
                    hs = slice(h * HD, (h + 1) * HD)
                    nc.tensor.matmul(
                        ps2[:, h, c0:TQT],
                        kT[hs, kb * P : (kb + 1) * P],
                        qT[hs, tq0 + c0 : tq0 + TQT],
                        start=True,
                        stop=True,
                    )
                et = e_pool.tile([P, 2, TQT], BF16, tag="et")
                nc.scalar.activation(
                    et[:, :, c0:TQT], ps2[:, :, c0:TQT], Exp, scale=SCALE
                )
                if m >= 0:
                    nc.vector.tensor_tensor(
                        et[:, :, c0 : c0 + P],
                        et[:, :, c0 : c0 + P],
                        cm_sb[:, m : m + 1, c0 : c0 + P].to_broadcast((P, 2, P)),
                        Mult,
                    )
                if DEBUG and g == 0 and qt == 0 and kb == 0:
                    nc.sync.dma_start(dbg["det"][:], et)
                for h in range(2):
                    nc.tensor.matmul(
                        po[h][:, c0:TQT],
                        va[:, kb, 2 * g + h, :],
                        et[:, h, c0:TQT],
                        start=(kb == 0),
                        stop=(kb == nblk - 1),
                    )
            # --- normalize: oT[hs, tq] = po[h][0:64] * (1/Z) ---
            # Z rows staged at physical partition 0: partition_broadcast
            # reads partition 0 of the source.
            for h in range(2):
                hs = slice(h * HD, (h + 1) * HD)
                zrow = z_pool.tile([1, TQT], F32, tag="zr", name=f"zr{h}")
                nc.vector.tensor_copy(zrow, po[h][HD : HD + 1, :])
                nc.vector.tensor_copy(oT[hs, tq0 : tq0 + TQT], po[h][0:HD, :])
                zrec = z_pool.tile([1, TQT], F32, tag="zc", name=f"zc{h}")
                nc.vector.reciprocal_approx_fast(zrec, zrow)
                rzb = z_pool.tile([P, TQT], F32, tag="rzb", name=f"rzb{h}")
                nc.gpsimd.partition_broadcast(rzb, zrec)
                nc.vector.tensor_tensor(
                    oT[hs, tq0 : tq0 + TQT],
                    oT[hs, tq0 : tq0 + TQT],
                    rzb[hs, :],
                    Mult,
                )
                if DEBUG and g == 0 and qt == 0:
                    nc.sync.dma_start(dbg["dz"][:][32 * h : 32 * h + 1, :], zrow)

        def stage_c_part(oTs, part):
            """Quarter of the output projection (token blocks 4*part..)."""
            for tn in range(part * (NBLK // 4), (part + 1) * (NBLK // 4)):
                for cc in range(D // TQT):
                    psy = psA.tile([P, TQT], F32, tag="psA")
                    for g in range(G):
                        nc.tensor.matmul(
                            psy,
                            oTs[g][:, tn * P : (tn + 1) * P],
                            wo_sb[:, g, cc * TQT : (cc + 1) * TQT],
                            start=(g == 0),
                            stop=(g == G - 1),
                        )
                    yt = y_pool.tile([P, TQT], BF16, tag="yt")
                    nc.vector.tensor_copy(yt, psy)
                    nc.sync.dma_start(
                        y_r[:, tn, cc * TQT : (cc + 1) * TQT], yt
                    )

        # ---- schedule ----
        for tn in range(NBLK):
            stage_v(tn)
        qkT = {0: stage_a(0)}
        oTs = {}
        if DEBUG:
            nc.sync.dma_start(dbg["dva"][:], va)
            nc.sync.dma_start(dbg["dq"][:], qkT[0][0])
            nc.sync.dma_start(dbg["dk"][:], qkT[0][1])
        for g in range(G):
            if g + 1 < G:
                qkT[g + 1] = stage_a(g + 1)
            oT = o_pool.tile([P, T], BF16, tag="oT", name=f"oT{g}")
            oTs[g] = oT
            qT, kT = qkT[g]
            for qt in range(T // TQT):
                stage_b_qt(g, qT, kT, oT, qt)
                if g == G - 1:
                    stage_c_part(oTs, qt)
            if DEBUG and g == 0:
                nc.sync.dma_start(dbg["do"][:], oT)
            del qkT[g]

    nc.compile()
    return nc


def make_core_inputs(x, wq, wk, wv, wo):
    """Host-side sharding/layout prep. Returns list of 8 in_maps."""
    x = np.asarray(x, dtype=np.float32)
    wq = np.asarray(wq, dtype=np.float32)
    wk = np.asarray(wk, dtype=np.float32)
    wv = np.asarray(wv, dtype=np.float32)
    wo = np.asarray(wo, dtype=np.float32)
    mdt = mybir.dt.np(BF16)

    xts = [np.ascontiguousarray(x[b].T).astype(mdt) for b in range(B)]
    i = np.arange(P)[:, None]
    j = np.arange(TQT)[None, :]
    cmask = np.stack(
        [(i + P * m <= j).astype(np.float32) for m in range(4)], axis=0
    )  # [4, P, TQT]

    in_maps = []
    for c in range(NCORES):
        b, hh = c // 2, c % 2
        js = slice(JD * hh, JD * (hh + 1))
        in_maps.append(
            {
                "xt": xts[b],
                "wq": np.ascontiguousarray(wq[js, :].T).astype(mdt),
                "wk": np.ascontiguousarray(wk[js, :].T).astype(mdt),
                "wv": np.ascontiguousarray(wv[js, :].T).astype(mdt),
                "wo": np.ascontiguousarray(wo[:, js].T).astype(mdt),
                "cmask": cmask,
            }
        )
    return in_maps


_CACHE = {}


def run(in_maps, **kwargs):
    from concourse.bass_utils import run_bass_kernel_spmd

    if "nc" not in _CACHE:
        _CACHE["nc"] = build_program()
    nc = _CACHE["nc"]
    res = run_bass_kernel_spmd(nc, in_maps, core_ids=list(range(NCORES)), **kwargs)
    return res


def kernel(x, wq, wk, wv, wo):
    in_maps = make_core_inputs(x, wq, wk, wv, wo)
    res = run(in_maps)
    y = np.zeros((B, T, D), dtype=np.float32)
    for c, r in enumerate(res.results):
        y[c // 2] += r["y"].astype(np.float32)
    return y


# revision 4
# speedup vs baseline: 1.0449x; 1.0449x over previous
"""Trainium2 Bass kernel v2: causal MHA (B=4, T=2048, D=1024, H=16).

Sharding: batch x head-half. Core c handles batch b=c//2 and heads
[8*hh, 8*hh+8) with hh=c%2 (512 of the 1024 q/k/v dims). It computes the
partial output y_c = attn(x_b; heads hh) @ wo[:, slice]^T; the full output
is y[b] = y_{2b} + y_{2b+1} (summed on host).

All matmul data is bf16 (fp32 PSUM accumulation). Per-core dataflow:
  x_b^T resident in SBUF as [128, ko=8, T]                    (4 MB)
  v_nat[tn]  = x_chunk^T-contract @ wv      [128 tok, 512 hd] -> va
               (v in natural [token, hd] layout: no PE transposes)
  qT_g,kT_g  = w_g @ x^T                    [128, T] per chunk g (2 heads)
  S^T block  = kT_blk^T-contract qT         [tk=128, tq<=512], both heads'
               S matmuls run concurrently via PE row-tiling (base 0/64)
  E = exp(S^T * scale)  (ACT), causal mask on diagonal blocks (DVE)
  PV: po[65, tq] += [v|1]^T-contract E      (row 64 = Z)
  normalize: Z rows -> recip_approx_fast -> gpsimd partition_broadcast ->
             fused (PSUM->SBUF) multiply into oT_g
  y tile = oT_g^T-contract @ woT (accum over g) -> bf16 -> DRAM
"""

import os
import numpy as np

import concourse.bass as bass
import concourse.bacc as bacc
import concourse.mybir as mybir
from concourse.tile import TileContext
from contextlib import ExitStack

B, T, D, H = 4, 2048, 1024, 16
HD = D // H            # 64 head dim
P = 128                # partitions
KO = D // P            # 8 contraction subtiles for projections
TQT = 512              # tq tile width
NBLK = T // P          # 16 tk blocks
HPC = 8                # heads per core
G = 4                  # head-chunks per core (2 heads each)
JD = HPC * HD          # 512 local q/k/v dims
NCORES = 8
SCALE = 1.0 / float(np.sqrt(np.float32(HD)))

F32 = mybir.dt.float32
BF16 = mybir.dt.bfloat16

Exp = mybir.ActivationFunctionType.Exp
Copy = mybir.ActivationFunctionType.Copy
Mult = mybir.AluOpType.mult

# how 1/Z reaches the oT normalize: broadcast straight from PSUM (fast path)
# or staged via ACT copies into 32-aligned SBUF rows (fallback)
Z_MODE = os.environ.get("BASS_Z_MODE", "act_sbuf")
DEBUG = os.environ.get("BASS_DEBUG", "0") == "1"


def build_program():
    nc = bacc.Bacc("TRN2", target_bir_lowering=False, num_devices=NCORES)
    xt = nc.dram_tensor("xt", [D, T], BF16, kind="ExternalInput")
    wq = nc.dram_tensor("wq", [D, JD], BF16, kind="ExternalInput")
    wk = nc.dram_tensor("wk", [D, JD], BF16, kind="ExternalInput")
    wv = nc.dram_tensor("wv", [D, JD], BF16, kind="ExternalInput")
    wo = nc.dram_tensor("wo", [JD, D], BF16, kind="ExternalInput")
    cm = nc.dram_tensor("cmask", [P, P], F32, kind="ExternalInput")
    y = nc.dram_tensor("y", [T, D], BF16, kind="ExternalOutput")
    if DEBUG:
        dq = nc.dram_tensor("dq", [P, T], BF16, kind="ExternalOutput")
        dk = nc.dram_tensor("dk", [P, T], BF16, kind="ExternalOutput")
        dva = nc.dram_tensor("dva", [P, NBLK, HPC, HD + 1], BF16,
                             kind="ExternalOutput")
        det = nc.dram_tensor("det", [P, 2, TQT], BF16, kind="ExternalOutput")
        dz = nc.dram_tensor("dz", [33, TQT], F32, kind="ExternalOutput")
        do = nc.dram_tensor("do", [P, T], BF16, kind="ExternalOutput")
    dbg = {"dq": dq, "dk": dk, "dva": dva, "det": det, "dz": dz,
           "do": do} if DEBUG else None

    xt_r = xt[:].rearrange("(ko p) t -> p ko t", p=P)
    y_r = y[:].rearrange("(tn p) c -> p tn c", p=P)

    with TileContext(nc) as tc, ExitStack() as ctx:
        const = ctx.enter_context(tc.tile_pool(name="const", bufs=1))
        va_pool = ctx.enter_context(tc.tile_pool(name="va", bufs=1))
        qk_pool = ctx.enter_context(tc.tile_pool(name="qk", bufs=2))
        o_pool = ctx.enter_context(tc.tile_pool(name="o", bufs=4))
        e_pool = ctx.enter_context(tc.tile_pool(name="e", bufs=3))
        z_pool = ctx.enter_context(tc.tile_pool(name="z", bufs=2))
        y_pool = ctx.enter_context(tc.tile_pool(name="yp", bufs=2))
        psA = ctx.enter_context(tc.tile_pool(name="psA", bufs=2, space="PSUM"))
        psS = ctx.enter_context(tc.tile_pool(name="psS", bufs=2, space="PSUM"))
        psO = ctx.enter_context(tc.tile_pool(name="psO", bufs=2, space="PSUM"))

        # --- constants into SBUF ---
        x_sb = const.tile([P, KO, T], BF16, tag="x")
        for tt in range(T // TQT):
            nc.sync.dma_start(
                x_sb[:, :, tt * TQT : (tt + 1) * TQT],
                xt_r[:, :, tt * TQT : (tt + 1) * TQT],
            )
        wq_sb = const.tile([P, KO, JD], BF16, tag="wq")
        wk_sb = const.tile([P, KO, JD], BF16, tag="wk")
        wv_sb = const.tile([P, KO, JD], BF16, tag="wv")
        for w_sb, w_d in ((wq_sb, wq), (wk_sb, wk), (wv_sb, wv)):
            nc.sync.dma_start(w_sb, w_d[:].rearrange("(ko p) j -> p ko j", p=P))
        wo_sb = const.tile([P, G, D], BF16, tag="wo")
        nc.sync.dma_start(wo_sb, wo[:].rearrange("(g p) i -> p g i", p=P))
        # additive causal mask for the in-diagonal 128x128 triangle:
        # 0 where key<=query, -1e30 above the diagonal (same for every m)
        cm_sb = const.tile([P, 1, P], F32, tag="cm")
        nc.sync.dma_start(cm_sb[:, 0, :], cm[:])

        # v in natural layout + shared ones column per (block, head)
        va = va_pool.tile([P, NBLK, HPC, HD + 1], BF16, tag="va")
        nc.vector.tensor_copy(
            va[:, :, :, HD : HD + 1],
            nc.const_aps.tensor(1.0, (P, NBLK, HPC, 1), F32),
        )

        def stage_v(tn):
            """v projection for token block tn, all 8 heads, natural layout."""
            psv = psA.tile([P, HPC, HD], F32, tag="psA", name=f"psv{tn}")
            for ko in range(KO):
                nc.tensor.matmul(
                    psv,
                    x_sb[:, ko, tn * P : (tn + 1) * P],
                    wv_sb[:, ko, :],
                    start=(ko == 0),
                    stop=(ko == KO - 1),
                )
            nc.vector.tensor_copy(va[:, tn, :, 0:HD], psv)

        def stage_a_pair(g, qT, kT, pp):
            """q/k projections for head-chunk g, token tiles 2pp, 2pp+1.
            The two tiles share each stationary weight chunk."""
            js = slice(g * P, (g + 1) * P)
            tts = (2 * pp, 2 * pp + 1)
            for w_sb, dst in ((wq_sb, qT), (wk_sb, kT)):
                pps = [
                    psA.tile([P, TQT], F32, tag="psA", name=f"pp{s}")
                    for s in range(2)
                ]
                for ko in range(KO):
                    for s in range(2):
                        ts = slice(tts[s] * TQT, (tts[s] + 1) * TQT)
                        nc.tensor.matmul(
                            pps[s],
                            w_sb[:, ko, js],
                            x_sb[:, ko, ts],
                            start=(ko == 0),
                            stop=(ko == KO - 1),
                        )
                for s in range(2):
                    ts = slice(tts[s] * TQT, (tts[s] + 1) * TQT)
                    nc.vector.tensor_copy(dst[:, ts], pps[s])

        def stage_a_single(g, qT, kT, tt):
            """q/k projection for one token tile (fastest first-S ramp)."""
            js = slice(g * P, (g + 1) * P)
            ts = slice(tt * TQT, (tt + 1) * TQT)
            for w_sb, dst in ((wq_sb, qT), (wk_sb, kT)):
                pp = psA.tile([P, TQT], F32, tag="psA", name=f"ps{tt}")
                for ko in range(KO):
                    nc.tensor.matmul(
                        pp,
                        w_sb[:, ko, js],
                        x_sb[:, ko, ts],
                        start=(ko == 0),
                        stop=(ko == KO - 1),
                    )
                nc.vector.tensor_copy(dst[:, ts], pp)

        def stage_b_qt(g, qT, kT, oT, qt):
            """Attention for head-chunk g, query tile qt."""
            tq0 = qt * TQT
            nblk = qt * 4 + 4
            po = [
                psO.tile([HD + 1, TQT], F32, tag="po", name=f"po{h}")
                for h in range(2)
            ]
            for kb in range(nblk):
                m = kb - qt * 4  # >=0: diagonal-crossing block
                c0 = P * m if m >= 0 else 0
                ps2 = psS.tile([P, 2, TQT], F32, tag="ps")
                for h in range(2):
                    hs = slice(h * HD, (h + 1) * HD)
                    nc.tensor.matmul(
                        ps2[:, h, c0:TQT],
                        kT[hs, kb * P : (kb + 1) * P],
                        qT[hs, tq0 + c0 : tq0 + TQT],
                        start=True,
                        stop=True,
                    )
                et = e_pool.tile([P, 2, TQT], BF16, tag="et")
                nc.scalar.activation(
                    et[:, :, c0:TQT], ps2[:, :, c0:TQT], Exp, scale=SCALE
                )
                if m >= 0:
                    nc.vector.tensor_tensor(
                        et[:, :, c0 : c0 + P],
                        et[:, :, c0 : c0 + P],
                        cm_sb[:, 0:1, :].to_broadcast((P, 2, P)),
                        Mult,
                    )
                if DEBUG and g == 0 and qt == 0 and kb == 0:
                    nc.sync.dma_start(dbg["det"][:], et)
                for h in range(2):
                    nc.tensor.matmul(
                        po[h][:, c0:TQT],
                        va[:, kb, 2 * g + h, :],
                        et[:, h, c0:TQT],
                        start=(kb == 0),
                        stop=(kb == nblk - 1),
                    )
            # --- normalize: oT[hs, tq] = po[h][0:64] * (1/Z) ---
            # Z rows staged at physical partition 0: partition_broadcast
            # reads partition 0 of the source.
            for h in range(2):
                hs = slice(h * HD, (h + 1) * HD)
                zrow = z_pool.tile([1, TQT], F32, tag="zr", name=f"zr{h}")
                nc.vector.tensor_copy(zrow, po[h][HD : HD + 1, :])
                zrec = z_pool.tile([1, TQT], F32, tag="zc", name=f"zc{h}")
                nc.vector.reciprocal_approx_fast(zrec, zrow)
                rzb = z_pool.tile([P, TQT], F32, tag="rzb", name=f"rzb{h}")
                nc.gpsimd.partition_broadcast(rzb, zrec)
                nc.vector.tensor_tensor(
                    oT[hs, tq0 : tq0 + TQT],
                    po[h][0:HD, :],
                    rzb[hs, :],
                    Mult,
                )
                if DEBUG and g == 0 and qt == 0:
                    nc.sync.dma_start(dbg["dz"][:][32 * h : 32 * h + 1, :], zrow)

        def stage_c_part(oTs, part):
            """Quarter of the output projection (token blocks 4*part..)."""
            for tn in range(part * (NBLK // 4), (part + 1) * (NBLK // 4)):
                for cc in range(D // TQT):
                    psy = psA.tile([P, TQT], F32, tag="psA")
                    for g in range(G):
                        nc.tensor.matmul(
                            psy,
                            oTs[g][:, tn * P : (tn + 1) * P],
                            wo_sb[:, g, cc * TQT : (cc + 1) * TQT],
                            start=(g == 0),
                            stop=(g == G - 1),
                        )
                    yt = y_pool.tile([P, TQT], BF16, tag="yt")
                    nc.vector.tensor_copy(yt, psy)
                    nc.sync.dma_start(
                        y_r[:, tn, cc * TQT : (cc + 1) * TQT], yt
                    )

        # ---- schedule ----
        # Ramp: interleave A(0) pairs, vnat chunks, and B(0, qt) so the
        # first exp lands ~20us in rather than after all projections.
        def new_qk(g):
            return (
                qk_pool.tile([P, T], BF16, tag="qT", name=f"qT{g}"),
                qk_pool.tile([P, T], BF16, tag="kT", name=f"kT{g}"),
            )

        qkT = {0: new_qk(0)}
        oTs = {}
        oTs[0] = o_pool.tile([P, T], BF16, tag="oT", name="oT0")
        for qt in range(T // TQT):
            stage_a_single(0, *qkT[0], qt)
            for tn in range(4 * qt, 4 * qt + 4):
                stage_v(tn)
            stage_b_qt(0, *qkT[0], oTs[0], qt)
        if DEBUG:
            nc.sync.dma_start(dbg["dva"][:], va)
            nc.sync.dma_start(dbg["dq"][:], qkT[0][0])
            nc.sync.dma_start(dbg["dk"][:], qkT[0][1])
            nc.sync.dma_start(dbg["do"][:], oTs[0])
        for g in range(1, G):
            qkT[g] = new_qk(g)
            for pp in range(2):
                stage_a_pair(g, *qkT[g], pp)
            del qkT[g - 1]
            oTs[g] = o_pool.tile([P, T], BF16, tag="oT", name=f"oT{g}")
            for qt in range(T // TQT):
                stage_b_qt(g, *qkT[g], oTs[g], qt)
                if g == G - 1:
                    stage_c_part(oTs, qt)

    nc.compile()
    return nc


def make_core_inputs(x, wq, wk, wv, wo):
    """Host-side sharding/layout prep. Returns list of 8 in_maps."""
    x = np.asarray(x, dtype=np.float32)
    wq = np.asarray(wq, dtype=np.float32)
    wk = np.asarray(wk, dtype=np.float32)
    wv = np.asarray(wv, dtype=np.float32)
    wo = np.asarray(wo, dtype=np.float32)
    mdt = mybir.dt.np(BF16)

    xts = [np.ascontiguousarray(x[b].T).astype(mdt) for b in range(B)]
    i = np.arange(P)[:, None]
    j = np.arange(P)[None, :]
    cmask = (i <= j).astype(np.float32)  # [P, P] multiplicative keep-mask

    in_maps = []
    for c in range(NCORES):
        b, hh = c // 2, c % 2
        js = slice(JD * hh, JD * (hh + 1))
        in_maps.append(
            {
                "xt": xts[b],
                "wq": np.ascontiguousarray(wq[js, :].T).astype(mdt),
                "wk": np.ascontiguousarray(wk[js, :].T).astype(mdt),
                "wv": np.ascontiguousarray(wv[js, :].T).astype(mdt),
                "wo": np.ascontiguousarray(wo[:, js].T).astype(mdt),
                "cmask": cmask,
            }
        )
    return in_maps


_CACHE = {}


def run(in_maps, **kwargs):
    from concourse.bass_utils import run_bass_kernel_spmd

    if "nc" not in _CACHE:
        _CACHE["nc"] = build_program()
    nc = _CACHE["nc"]
    res = run_bass_kernel_spmd(nc, in_maps, core_ids=list(range(NCORES)), **kwargs)
    return res


def kernel(x, wq, wk, wv, wo):
    in_maps = make_core_inputs(x, wq, wk, wv, wo)
    res = run(in_maps)
    y = np.zeros((B, T, D), dtype=np.float32)
    for c, r in enumerate(res.results):
        y[c // 2] += r["y"].astype(np.float32)
    return y


# revision 5
# speedup vs baseline: 1.0539x; 1.0087x over previous
"""Trainium2 Bass kernel v2: causal MHA (B=4, T=2048, D=1024, H=16).

Sharding: batch x head-half. Core c handles batch b=c//2 and heads
[8*hh, 8*hh+8) with hh=c%2 (512 of the 1024 q/k/v dims). It computes the
partial output y_c = attn(x_b; heads hh) @ wo[:, slice]^T; the full output
is y[b] = y_{2b} + y_{2b+1} (summed on host).

All matmul data is bf16 (fp32 PSUM accumulation). Per-core dataflow:
  x_b^T resident in SBUF as [128, ko=8, T]                    (4 MB)
  v_nat[tn]  = x_chunk^T-contract @ wv      [128 tok, 512 hd] -> va
               (v in natural [token, hd] layout: no PE transposes)
  qT_g,kT_g  = w_g @ x^T                    [128, T] per chunk g (2 heads)
  S^T block  = kT_blk^T-contract qT         [tk=128, tq<=512], both heads'
               S matmuls run concurrently via PE row-tiling (base 0/64)
  E = exp(S^T * scale)  (ACT), causal mask on diagonal blocks (DVE)
  PV: po[65, tq] += [v|1]^T-contract E      (row 64 = Z)
  normalize: Z rows -> recip_approx_fast -> gpsimd partition_broadcast ->
             fused (PSUM->SBUF) multiply into oT_g
  y tile = oT_g^T-contract @ woT (accum over g) -> bf16 -> DRAM
"""

import os
import numpy as np

import concourse.bass as bass
import concourse.bacc as bacc
import concourse.mybir as mybir
from concourse.tile import TileContext
from contextlib import ExitStack

B, T, D, H = 4, 2048, 1024, 16
HD = D // H            # 64 head dim
P = 128                # partitions
KO = D // P            # 8 contraction subtiles for projections
TQT = 512              # tq tile width
NBLK = T // P          # 16 tk blocks
HPC = 8                # heads per core
G = 4                  # head-chunks per core (2 heads each)
JD = HPC * HD          # 512 local q/k/v dims
NCORES = 8
SCALE = 1.0 / float(np.sqrt(np.float32(HD)))

F32 = mybir.dt.float32
BF16 = mybir.dt.bfloat16

Exp = mybir.ActivationFunctionType.Exp
Copy = mybir.ActivationFunctionType.Copy
Mult = mybir.AluOpType.mult

# how 1/Z reaches the oT normalize: broadcast straight from PSUM (fast path)
# or staged via ACT copies into 32-aligned SBUF rows (fallback)
Z_MODE = os.environ.get("BASS_Z_MODE", "act_sbuf")
DEBUG = os.environ.get("BASS_DEBUG", "0") == "1"


def build_program():
    nc = bacc.Bacc("TRN2", target_bir_lowering=False, num_devices=NCORES)
    xt = nc.dram_tensor("xt", [D, T], BF16, kind="ExternalInput")
    wq = nc.dram_tensor("wq", [D, JD], BF16, kind="ExternalInput")
    wk = nc.dram_tensor("wk", [D, JD], BF16, kind="ExternalInput")
    wv = nc.dram_tensor("wv", [D, JD], BF16, kind="ExternalInput")
    wo = nc.dram_tensor("wo", [JD, D], BF16, kind="ExternalInput")
    cm = nc.dram_tensor("cmask", [P, P], F32, kind="ExternalInput")
    y = nc.dram_tensor("y", [T, D], BF16, kind="ExternalOutput")
    if DEBUG:
        dq = nc.dram_tensor("dq", [P, T], BF16, kind="ExternalOutput")
        dk = nc.dram_tensor("dk", [P, T], BF16, kind="ExternalOutput")
        dva = nc.dram_tensor("dva", [P, NBLK, HPC, HD + 1], BF16,
                             kind="ExternalOutput")
        det = nc.dram_tensor("det", [P, 2, TQT], BF16, kind="ExternalOutput")
        dz = nc.dram_tensor("dz", [33, TQT], F32, kind="ExternalOutput")
        do = nc.dram_tensor("do", [P, T], BF16, kind="ExternalOutput")
    dbg = {"dq": dq, "dk": dk, "dva": dva, "det": det, "dz": dz,
           "do": do} if DEBUG else None

    xt_r = xt[:].rearrange("(ko p) t -> p ko t", p=P)
    y_r = y[:].rearrange("(tn p) c -> p tn c", p=P)

    with TileContext(nc) as tc, ExitStack() as ctx:
        const = ctx.enter_context(tc.tile_pool(name="const", bufs=1))
        va_pool = ctx.enter_context(tc.tile_pool(name="va", bufs=1))
        qk_pool = ctx.enter_context(tc.tile_pool(name="qk", bufs=2))
        o_pool = ctx.enter_context(tc.tile_pool(name="o", bufs=4))
        e_pool = ctx.enter_context(tc.tile_pool(name="e", bufs=3))
        z_pool = ctx.enter_context(tc.tile_pool(name="z", bufs=2))
        y_pool = ctx.enter_context(tc.tile_pool(name="yp", bufs=2))
        psA = ctx.enter_context(tc.tile_pool(name="psA", bufs=2, space="PSUM"))
        psS = ctx.enter_context(tc.tile_pool(name="psS", bufs=2, space="PSUM"))
        psO = ctx.enter_context(tc.tile_pool(name="psO", bufs=2, space="PSUM"))

        # --- constants into SBUF ---
        x_sb = const.tile([P, KO, T], BF16, tag="x")

        def load_x(tt):
            nc.sync.dma_start(
                x_sb[:, :, tt * TQT : (tt + 1) * TQT],
                xt_r[:, :, tt * TQT : (tt + 1) * TQT],
            )

        load_x(0)
        wq_sb = const.tile([P, KO, JD], BF16, tag="wq")
        wk_sb = const.tile([P, KO, JD], BF16, tag="wk")
        wv_sb = const.tile([P, KO, JD], BF16, tag="wv")
        for w_sb, w_d in ((wq_sb, wq), (wk_sb, wk), (wv_sb, wv)):
            nc.sync.dma_start(w_sb, w_d[:].rearrange("(ko p) j -> p ko j", p=P))
        wo_sb = const.tile([P, G, D], BF16, tag="wo")
        # additive causal mask for the in-diagonal 128x128 triangle:
        # 0 where key<=query, -1e30 above the diagonal (same for every m)
        cm_sb = const.tile([P, 1, P], F32, tag="cm")
        nc.sync.dma_start(cm_sb[:, 0, :], cm[:])

        # v in natural layout + shared ones column per (block, head)
        va = va_pool.tile([P, NBLK, HPC, HD + 1], BF16, tag="va")
        nc.vector.tensor_copy(
            va[:, :, :, HD : HD + 1],
            nc.const_aps.tensor(1.0, (P, NBLK, HPC, 1), F32),
        )

        def stage_v(tn):
            """v projection for token block tn, all 8 heads, natural layout."""
            psv = psA.tile([P, HPC, HD], F32, tag="psA", name=f"psv{tn}")
            for ko in range(KO):
                nc.tensor.matmul(
                    psv,
                    x_sb[:, ko, tn * P : (tn + 1) * P],
                    wv_sb[:, ko, :],
                    start=(ko == 0),
                    stop=(ko == KO - 1),
                )
            nc.vector.tensor_copy(va[:, tn, :, 0:HD], psv)

        def stage_a_pair(g, qT, kT, pp):
            """q/k projections for head-chunk g, token tiles 2pp, 2pp+1.
            The two tiles share each stationary weight chunk."""
            js = slice(g * P, (g + 1) * P)
            tts = (2 * pp, 2 * pp + 1)
            for w_sb, dst in ((wq_sb, qT), (wk_sb, kT)):
                pps = [
                    psA.tile([P, TQT], F32, tag="psA", name=f"pp{s}")
                    for s in range(2)
                ]
                for ko in range(KO):
                    for s in range(2):
                        ts = slice(tts[s] * TQT, (tts[s] + 1) * TQT)
                        nc.tensor.matmul(
                            pps[s],
                            w_sb[:, ko, js],
                            x_sb[:, ko, ts],
                            start=(ko == 0),
                            stop=(ko == KO - 1),
                        )
                for s in range(2):
                    ts = slice(tts[s] * TQT, (tts[s] + 1) * TQT)
                    nc.vector.tensor_copy(dst[:, ts], pps[s])

        def stage_a_single(g, qT, kT, tt):
            """q/k projection for one token tile (fastest first-S ramp)."""
            js = slice(g * P, (g + 1) * P)
            ts = slice(tt * TQT, (tt + 1) * TQT)
            for w_sb, dst in ((wq_sb, qT), (wk_sb, kT)):
                pp = psA.tile([P, TQT], F32, tag="psA", name=f"ps{tt}")
                for ko in range(KO):
                    nc.tensor.matmul(
                        pp,
                        w_sb[:, ko, js],
                        x_sb[:, ko, ts],
                        start=(ko == 0),
                        stop=(ko == KO - 1),
                    )
                nc.vector.tensor_copy(dst[:, ts], pp)

        def stage_b_qt(g, qT, kT, oT, qt):
            """Attention for head-chunk g, query tile qt."""
            tq0 = qt * TQT
            nblk = qt * 4 + 4
            po = [
                psO.tile([HD + 1, TQT], F32, tag="po", name=f"po{h}")
                for h in range(2)
            ]
            for kb in range(nblk):
                m = kb - qt * 4  # >=0: diagonal-crossing block
                c0 = P * m if m >= 0 else 0
                ps2 = psS.tile([P, 2, TQT], F32, tag="ps")
                for h in range(2):
                    hs = slice(h * HD, (h + 1) * HD)
                    nc.tensor.matmul(
                        ps2[:, h, c0:TQT],
                        kT[hs, kb * P : (kb + 1) * P],
                        qT[hs, tq0 + c0 : tq0 + TQT],
                        start=True,
                        stop=True,
                    )
                et = e_pool.tile([P, 2, TQT], BF16, tag="et")
                nc.scalar.activation(
                    et[:, :, c0:TQT], ps2[:, :, c0:TQT], Exp, scale=SCALE
                )
                if m >= 0:
                    nc.vector.tensor_tensor(
                        et[:, :, c0 : c0 + P],
                        et[:, :, c0 : c0 + P],
                        cm_sb[:, 0:1, :].to_broadcast((P, 2, P)),
                        Mult,
                    )
                if DEBUG and g == 0 and qt == 0 and kb == 0:
                    nc.sync.dma_start(dbg["det"][:], et)
                for h in range(2):
                    nc.tensor.matmul(
                        po[h][:, c0:TQT],
                        va[:, kb, 2 * g + h, :],
                        et[:, h, c0:TQT],
                        start=(kb == 0),
                        stop=(kb == nblk - 1),
                    )
            # --- normalize: oT[hs, tq] = po[h][0:64] * (1/Z) ---
            # Z rows staged at physical partition 0: partition_broadcast
            # reads partition 0 of the source.
            for h in range(2):
                hs = slice(h * HD, (h + 1) * HD)
                zrow = z_pool.tile([1, TQT], F32, tag="zr", name=f"zr{h}")
                nc.vector.tensor_copy(zrow, po[h][HD : HD + 1, :])
                zrec = z_pool.tile([1, TQT], F32, tag="zc", name=f"zc{h}")
                nc.vector.reciprocal_approx_fast(zrec, zrow)
                rzb = z_pool.tile([P, TQT], F32, tag="rzb", name=f"rzb{h}")
                nc.gpsimd.partition_broadcast(rzb, zrec)
                nc.vector.tensor_tensor(
                    oT[hs, tq0 : tq0 + TQT],
                    po[h][0:HD, :],
                    rzb[hs, :],
                    Mult,
                )
                if DEBUG and g == 0 and qt == 0:
                    nc.sync.dma_start(dbg["dz"][:][32 * h : 32 * h + 1, :], zrow)

        def stage_c_part(oTs, part):
            """Quarter of the output projection (token blocks 4*part..)."""
            for tn in range(part * (NBLK // 4), (part + 1) * (NBLK // 4)):
                for cc in range(D // TQT):
                    psy = psA.tile([P, TQT], F32, tag="psA")
                    for g in range(G):
                        nc.tensor.matmul(
                            psy,
                            oTs[g][:, tn * P : (tn + 1) * P],
                            wo_sb[:, g, cc * TQT : (cc + 1) * TQT],
                            start=(g == 0),
                            stop=(g == G - 1),
                        )
                    yt = y_pool.tile([P, TQT], BF16, tag="yt")
                    nc.vector.tensor_copy(yt, psy)
                    nc.sync.dma_start(
                        y_r[:, tn, cc * TQT : (cc + 1) * TQT], yt
                    )

        # ---- schedule ----
        # Ramp: interleave A(0) pairs, vnat chunks, and B(0, qt) so the
        # first exp lands ~20us in rather than after all projections.
        def new_qk(g):
            return (
                qk_pool.tile([P, T], BF16, tag="qT", name=f"qT{g}"),
                qk_pool.tile([P, T], BF16, tag="kT", name=f"kT{g}"),
            )

        qkT = {0: new_qk(0)}
        oTs = {}
        oTs[0] = o_pool.tile([P, T], BF16, tag="oT", name="oT0")
        for qt in range(T // TQT):
            if qt + 1 < T // TQT:
                load_x(qt + 1)
            stage_a_single(0, *qkT[0], qt)
            for tn in range(4 * qt, 4 * qt + 4):
                stage_v(tn)
            stage_b_qt(0, *qkT[0], oTs[0], qt)
        nc.sync.dma_start(wo_sb, wo[:].rearrange("(g p) i -> p g i", p=P))
        if DEBUG:
            nc.sync.dma_start(dbg["dva"][:], va)
            nc.sync.dma_start(dbg["dq"][:], qkT[0][0])
            nc.sync.dma_start(dbg["dk"][:], qkT[0][1])
            nc.sync.dma_start(dbg["do"][:], oTs[0])
        for g in range(1, G):
            qkT[g] = new_qk(g)
            for pp in range(2):
                stage_a_pair(g, *qkT[g], pp)
            del qkT[g - 1]
            oTs[g] = o_pool.tile([P, T], BF16, tag="oT", name=f"oT{g}")
            for qt in range(T // TQT):
                stage_b_qt(g, *qkT[g], oTs[g], qt)
                if g == G - 1:
                    stage_c_part(oTs, qt)

    nc.compile()
    return nc


def make_core_inputs(x, wq, wk, wv, wo):
    """Host-side sharding/layout prep. Returns list of 8 in_maps."""
    x = np.asarray(x, dtype=np.float32)
    wq = np.asarray(wq, dtype=np.float32)
    wk = np.asarray(wk, dtype=np.float32)
    wv = np.asarray(wv, dtype=np.float32)
    wo = np.asarray(wo, dtype=np.float32)
    mdt = mybir.dt.np(BF16)

    xts = [np.ascontiguousarray(x[b].T).astype(mdt) for b in range(B)]
    i = np.arange(P)[:, None]
    j = np.arange(P)[None, :]
    cmask = (i <= j).astype(np.float32)  # [P, P] multiplicative keep-mask

    in_maps = []
    for c in range(NCORES):
        b, hh = c // 2, c % 2
        js = slice(JD * hh, JD * (hh + 1))
        in_maps.append(
            {
                "xt": xts[b],
                "wq": np.ascontiguousarray(wq[js, :].T).astype(mdt),
                "wk": np.ascontiguousarray(wk[js, :].T).astype(mdt),
                "wv": np.ascontiguousarray(wv[js, :].T).astype(mdt),
                "wo": np.ascontiguousarray(wo[:, js].T).astype(mdt),
                "cmask": cmask,
            }
        )
    return in_maps


_CACHE = {}


def run(in_maps, **kwargs):
    from concourse.bass_utils import run_bass_kernel_spmd

    if "nc" not in _CACHE:
        _CACHE["nc"] = build_program()
    nc = _CACHE["nc"]
    res = run_bass_kernel_spmd(nc, in_maps, core_ids=list(range(NCORES)), **kwargs)
    return res


def kernel(x, wq, wk, wv, wo):
    in_maps = make_core_inputs(x, wq, wk, wv, wo)
    res = run(in_maps)
    y = np.zeros((B, T, D), dtype=np.float32)
    for c, r in enumerate(res.results):
        y[c // 2] += r["y"].astype(np.float32)
    return y


# revision 6
# speedup vs baseline: 1.0627x; 1.0083x over previous
"""Trainium2 Bass kernel v2: causal MHA (B=4, T=2048, D=1024, H=16).

Sharding: batch x head-half. Core c handles batch b=c//2 and heads
[8*hh, 8*hh+8) with hh=c%2 (512 of the 1024 q/k/v dims). It computes the
partial output y_c = attn(x_b; heads hh) @ wo[:, slice]^T; the full output
is y[b] = y_{2b} + y_{2b+1} (summed on host).

All matmul data is bf16 (fp32 PSUM accumulation). Per-core dataflow:
  x_b^T resident in SBUF as [128, ko=8, T]                    (4 MB)
  v_nat[tn]  = x_chunk^T-contract @ wv      [128 tok, 512 hd] -> va
               (v in natural [token, hd] layout: no PE transposes)
  qT_g,kT_g  = w_g @ x^T                    [128, T] per chunk g (2 heads)
  S^T block  = kT_blk^T-contract qT         [tk=128, tq<=512], both heads'
               S matmuls run concurrently via PE row-tiling (base 0/64)
  E = exp(S^T * scale)  (ACT), causal mask on diagonal blocks (DVE)
  PV: po[65, tq] += [v|1]^T-contract E      (row 64 = Z)
  normalize: Z rows -> recip_approx_fast -> gpsimd partition_broadcast ->
             fused (PSUM->SBUF) multiply into oT_g
  y tile = oT_g^T-contract @ woT (accum over g) -> bf16 -> DRAM
"""

import os
import numpy as np

import concourse.bass as bass
import concourse.bacc as bacc
import concourse.mybir as mybir
from concourse.tile import TileContext
from contextlib import ExitStack

B, T, D, H = 4, 2048, 1024, 16
HD = D // H            # 64 head dim
P = 128                # partitions
KO = D // P            # 8 contraction subtiles for projections
TQT = 512              # tq tile width
NBLK = T // P          # 16 tk blocks
HPC = 8                # heads per core
G = 4                  # head-chunks per core (2 heads each)
JD = HPC * HD          # 512 local q/k/v dims
NCORES = 8
SCALE = 1.0 / float(np.sqrt(np.float32(HD)))

F32 = mybir.dt.float32
BF16 = mybir.dt.bfloat16

Exp = mybir.ActivationFunctionType.Exp
Copy = mybir.ActivationFunctionType.Copy
Mult = mybir.AluOpType.mult

# how 1/Z reaches the oT normalize: broadcast straight from PSUM (fast path)
# or staged via ACT copies into 32-aligned SBUF rows (fallback)
Z_MODE = os.environ.get("BASS_Z_MODE", "act_sbuf")
DEBUG = os.environ.get("BASS_DEBUG", "0") == "1"


def build_program():
    nc = bacc.Bacc("TRN2", target_bir_lowering=False, num_devices=NCORES)
    xt = nc.dram_tensor("xt", [D, T], BF16, kind="ExternalInput")
    wq = nc.dram_tensor("wq", [D, JD], BF16, kind="ExternalInput")
    wk = nc.dram_tensor("wk", [D, JD], BF16, kind="ExternalInput")
    wv = nc.dram_tensor("wv", [D, JD], BF16, kind="ExternalInput")
    wo = nc.dram_tensor("wo", [JD, D], BF16, kind="ExternalInput")
    cm = nc.dram_tensor("cmask", [P, P], F32, kind="ExternalInput")
    y = nc.dram_tensor("y", [T, D], BF16, kind="ExternalOutput")
    if DEBUG:
        dq = nc.dram_tensor("dq", [P, T], BF16, kind="ExternalOutput")
        dk = nc.dram_tensor("dk", [P, T], BF16, kind="ExternalOutput")
        dva = nc.dram_tensor("dva", [P, NBLK, HPC, HD + 1], BF16,
                             kind="ExternalOutput")
        det = nc.dram_tensor("det", [P, 2, TQT], BF16, kind="ExternalOutput")
        dz = nc.dram_tensor("dz", [33, TQT], F32, kind="ExternalOutput")
        do = nc.dram_tensor("do", [P, T], BF16, kind="ExternalOutput")
    dbg = {"dq": dq, "dk": dk, "dva": dva, "det": det, "dz": dz,
           "do": do} if DEBUG else None

    xt_r = xt[:].rearrange("(ko p) t -> p ko t", p=P)
    y_r = y[:].rearrange("(tn p) c -> p tn c", p=P)

    with TileContext(nc) as tc, ExitStack() as ctx:
        const = ctx.enter_context(tc.tile_pool(name="const", bufs=1))
        va_pool = ctx.enter_context(tc.tile_pool(name="va", bufs=1))
        qk_pool = ctx.enter_context(tc.tile_pool(name="qk", bufs=4))
        o_pool = ctx.enter_context(tc.tile_pool(name="o", bufs=4))
        e_pool = ctx.enter_context(tc.tile_pool(name="e", bufs=3))
        z_pool = ctx.enter_context(tc.tile_pool(name="z", bufs=2))
        y_pool = ctx.enter_context(tc.tile_pool(name="yp", bufs=2))
        psA = ctx.enter_context(tc.tile_pool(name="psA", bufs=2, space="PSUM"))
        psS = ctx.enter_context(tc.tile_pool(name="psS", bufs=2, space="PSUM"))
        psO = ctx.enter_context(tc.tile_pool(name="psO", bufs=2, space="PSUM"))

        # --- constants into SBUF ---
        x_sb = const.tile([P, KO, T], BF16, tag="x")

        def load_x(tt):
            nc.sync.dma_start(
                x_sb[:, :, tt * TQT : (tt + 1) * TQT],
                xt_r[:, :, tt * TQT : (tt + 1) * TQT],
            )

        load_x(0)
        wq_sb = const.tile([P, KO, JD], BF16, tag="wq")
        wk_sb = const.tile([P, KO, JD], BF16, tag="wk")
        wv_sb = const.tile([P, KO, JD], BF16, tag="wv")
        for w_sb, w_d in ((wq_sb, wq), (wk_sb, wk), (wv_sb, wv)):
            nc.sync.dma_start(w_sb, w_d[:].rearrange("(ko p) j -> p ko j", p=P))
        wo_sb = const.tile([P, G, D], BF16, tag="wo")
        # additive causal mask for the in-diagonal 128x128 triangle:
        # 0 where key<=query, -1e30 above the diagonal (same for every m)
        cm_sb = const.tile([P, 1, P], F32, tag="cm")
        nc.sync.dma_start(cm_sb[:, 0, :], cm[:])

        # v in natural layout + shared ones column per (block, head)
        va = va_pool.tile([P, NBLK, HPC, HD + 1], BF16, tag="va")
        nc.vector.tensor_copy(
            va[:, :, :, HD : HD + 1],
            nc.const_aps.tensor(1.0, (P, NBLK, HPC, 1), F32),
        )

        def stage_v(tn):
            """v projection for token block tn, all 8 heads, natural layout."""
            psv = psA.tile([P, HPC, HD], F32, tag="psA", name=f"psv{tn}")
            for ko in range(KO):
                nc.tensor.matmul(
                    psv,
                    x_sb[:, ko, tn * P : (tn + 1) * P],
                    wv_sb[:, ko, :],
                    start=(ko == 0),
                    stop=(ko == KO - 1),
                )
            nc.vector.tensor_copy(va[:, tn, :, 0:HD], psv)

        def stage_a_pair(g, qT, kT, pp):
            """q/k projections for head-chunk g, token tiles 2pp, 2pp+1.
            The two tiles share each stationary weight chunk."""
            js = slice(g * P, (g + 1) * P)
            tts = (2 * pp, 2 * pp + 1)
            for w_sb, dst in ((wq_sb, qT), (wk_sb, kT)):
                pps = [
                    psA.tile([P, TQT], F32, tag="psA", name=f"pp{s}")
                    for s in range(2)
                ]
                for ko in range(KO):
                    for s in range(2):
                        ts = slice(tts[s] * TQT, (tts[s] + 1) * TQT)
                        nc.tensor.matmul(
                            pps[s],
                            w_sb[:, ko, js],
                            x_sb[:, ko, ts],
                            start=(ko == 0),
                            stop=(ko == KO - 1),
                        )
                for s in range(2):
                    ts = slice(tts[s] * TQT, (tts[s] + 1) * TQT)
                    nc.vector.tensor_copy(dst[:, ts], pps[s])

        def stage_a_single(g, qT, kT, tt):
            """q/k projection for one token tile (fastest first-S ramp)."""
            js = slice(g * P, (g + 1) * P)
            ts = slice(tt * TQT, (tt + 1) * TQT)
            for w_sb, dst in ((wq_sb, qT), (wk_sb, kT)):
                pp = psA.tile([P, TQT], F32, tag="psA", name=f"ps{tt}")
                for ko in range(KO):
                    nc.tensor.matmul(
                        pp,
                        w_sb[:, ko, js],
                        x_sb[:, ko, ts],
                        start=(ko == 0),
                        stop=(ko == KO - 1),
                    )
                nc.vector.tensor_copy(dst[:, ts], pp)

        def stage_b_qt(g, qT, kT, oT, qt):
            """Attention for head-chunk g, query tile qt."""
            tq0 = qt * TQT
            nblk = qt * 4 + 4
            po = [
                psO.tile([HD + 1, TQT], F32, tag="po", name=f"po{h}")
                for h in range(2)
            ]
            for kb in range(nblk):
                m = kb - qt * 4  # >=0: diagonal-crossing block
                c0 = P * m if m >= 0 else 0
                ps2 = psS.tile([P, 2, TQT], F32, tag="ps")
                for h in range(2):
                    hs = slice(h * HD, (h + 1) * HD)
                    nc.tensor.matmul(
                        ps2[:, h, c0:TQT],
                        kT[hs, kb * P : (kb + 1) * P],
                        qT[hs, tq0 + c0 : tq0 + TQT],
                        start=True,
                        stop=True,
                    )
                et = e_pool.tile([P, 2, TQT], BF16, tag="et")
                nc.scalar.activation(
                    et[:, :, c0:TQT], ps2[:, :, c0:TQT], Exp, scale=SCALE
                )
                if m >= 0:
                    nc.vector.tensor_tensor(
                        et[:, :, c0 : c0 + P],
                        et[:, :, c0 : c0 + P],
                        cm_sb[:, 0:1, :].to_broadcast((P, 2, P)),
                        Mult,
                    )
                if DEBUG and g == 0 and qt == 0 and kb == 0:
                    nc.sync.dma_start(dbg["det"][:], et)
                for h in range(2):
                    nc.tensor.matmul(
                        po[h][:, c0:TQT],
                        va[:, kb, 2 * g + h, :],
                        et[:, h, c0:TQT],
                        start=(kb == 0),
                        stop=(kb == nblk - 1),
                    )
            # --- normalize: oT[hs, tq] = po[h][0:64] * (1/Z) ---
            # Z rows staged at physical partition 0: partition_broadcast
            # reads partition 0 of the source.
            for h in range(2):
                hs = slice(h * HD, (h + 1) * HD)
                zrow = z_pool.tile([1, TQT], F32, tag="zr", name=f"zr{h}")
                nc.vector.tensor_copy(zrow, po[h][HD : HD + 1, :])
                zrec = z_pool.tile([1, TQT], F32, tag="zc", name=f"zc{h}")
                nc.vector.reciprocal_approx_fast(zrec, zrow)
                rzb = z_pool.tile([P, TQT], F32, tag="rzb", name=f"rzb{h}")
                nc.gpsimd.partition_broadcast(rzb, zrec)
                nc.vector.tensor_tensor(
                    oT[hs, tq0 : tq0 + TQT],
                    po[h][0:HD, :],
                    rzb[hs, :],
                    Mult,
                )
                if DEBUG and g == 0 and qt == 0:
                    nc.sync.dma_start(dbg["dz"][:][32 * h : 32 * h + 1, :], zrow)

        def stage_c_part(oTs, part):
            """Quarter of the output projection (token blocks 4*part..)."""
            for tn in range(part * (NBLK // 4), (part + 1) * (NBLK // 4)):
                for cc in range(D // TQT):
                    psy = psA.tile([P, TQT], F32, tag="psA")
                    for g in range(G):
                        nc.tensor.matmul(
                            psy,
                            oTs[g][:, tn * P : (tn + 1) * P],
                            wo_sb[:, g, cc * TQT : (cc + 1) * TQT],
                            start=(g == 0),
                            stop=(g == G - 1),
                        )
                    yt = y_pool.tile([P, TQT], BF16, tag="yt")
                    nc.vector.tensor_copy(yt, psy)
                    nc.sync.dma_start(
                        y_r[:, tn, cc * TQT : (cc + 1) * TQT], yt
                    )

        # ---- schedule ----
        # qt-major sweeps: every sweep does vnat chunk, all 4 head-chunks'
        # JIT projection + attention for this query tile, then the previous
        # tile's output projection. Balances vnat/proj/C across the run.
        def new_qk(g):
            return (
                qk_pool.tile([P, T], BF16, tag="qT", name=f"qT{g}"),
                qk_pool.tile([P, T], BF16, tag="kT", name=f"kT{g}"),
            )

        qkT = {g: new_qk(g) for g in range(G)}
        oTs = {
            g: o_pool.tile([P, T], BF16, tag="oT", name=f"oT{g}")
            for g in range(G)
        }
        for qt in range(T // TQT):
            if qt + 1 < T // TQT:
                load_x(qt + 1)
            if qt == 1:
                nc.sync.dma_start(
                    wo_sb, wo[:].rearrange("(g p) i -> p g i", p=P)
                )
            for tn in range(4 * qt, 4 * qt + 4):
                stage_v(tn)
            for g in range(G):
                stage_a_single(g, *qkT[g], qt)
                stage_b_qt(g, *qkT[g], oTs[g], qt)
            if qt > 0:
                stage_c_part(oTs, qt - 1)
        stage_c_part(oTs, T // TQT - 1)
        if DEBUG:
            nc.sync.dma_start(dbg["dva"][:], va)
            nc.sync.dma_start(dbg["dq"][:], qkT[0][0])
            nc.sync.dma_start(dbg["dk"][:], qkT[0][1])
            nc.sync.dma_start(dbg["do"][:], oTs[0])

    nc.compile()
    return nc


def make_core_inputs(x, wq, wk, wv, wo):
    """Host-side sharding/layout prep. Returns list of 8 in_maps."""
    x = np.asarray(x, dtype=np.float32)
    wq = np.asarray(wq, dtype=np.float32)
    wk = np.asarray(wk, dtype=np.float32)
    wv = np.asarray(wv, dtype=np.float32)
    wo = np.asarray(wo, dtype=np.float32)
    mdt = mybir.dt.np(BF16)

    xts = [np.ascontiguousarray(x[b].T).astype(mdt) for b in range(B)]
    i = np.arange(P)[:, None]
    j = np.arange(P)[None, :]
    cmask = (i <= j).astype(np.float32)  # [P, P] multiplicative keep-mask

    in_maps = []
    for c in range(NCORES):
        b, hh = c // 2, c % 2
        js = slice(JD * hh, JD * (hh + 1))
        in_maps.append(
            {
                "xt": xts[b],
                "wq": np.ascontiguousarray(wq[js, :].T).astype(mdt),
                "wk": np.ascontiguousarray(wk[js, :].T).astype(mdt),
                "wv": np.ascontiguousarray(wv[js, :].T).astype(mdt),
                "wo": np.ascontiguousarray(wo[:, js].T).astype(mdt),
                "cmask": cmask,
            }
        )
    return in_maps


_CACHE = {}


def run(in_maps, **kwargs):
    from concourse.bass_utils import run_bass_kernel_spmd

    if "nc" not in _CACHE:
        _CACHE["nc"] = build_program()
    nc = _CACHE["nc"]
    res = run_bass_kernel_spmd(nc, in_maps, core_ids=list(range(NCORES)), **kwargs)
    return res


def kernel(x, wq, wk, wv, wo):
    in_maps = make_core_inputs(x, wq, wk, wv, wo)
    res = run(in_maps)
    y = np.zeros((B, T, D), dtype=np.float32)
    for c, r in enumerate(res.results):
        y[c // 2] += r["y"].astype(np.float32)
    return y


# revision 7
# speedup vs baseline: 1.0932x; 1.0287x over previous
"""Trainium2 Bass kernel v2: causal MHA (B=4, T=2048, D=1024, H=16).

Sharding: batch x head-half. Core c handles batch b=c//2 and heads
[8*hh, 8*hh+8) with hh=c%2 (512 of the 1024 q/k/v dims). It computes the
partial output y_c = attn(x_b; heads hh) @ wo[:, slice]^T; the full output
is y[b] = y_{2b} + y_{2b+1} (summed on host).

All matmul data is bf16 (fp32 PSUM accumulation). Per-core dataflow:
  x_b^T resident in SBUF as [128, ko=8, T]                    (4 MB)
  v_nat[tn]  = x_chunk^T-contract @ wv      [128 tok, 512 hd] -> va
               (v in natural [token, hd] layout: no PE transposes)
  qT_g,kT_g  = w_g @ x^T                    [128, T] per chunk g (2 heads)
  S^T block  = kT_blk^T-contract qT         [tk=128, tq<=512], both heads'
               S matmuls run concurrently via PE row-tiling (base 0/64)
  E = exp(S^T * scale)  (ACT), causal mask on diagonal blocks (DVE)
  PV: po[65, tq] += [v|1]^T-contract E      (row 64 = Z)
  normalize: Z rows -> recip_approx_fast -> gpsimd partition_broadcast ->
             fused (PSUM->SBUF) multiply into oT_g
  y tile = oT_g^T-contract @ woT (accum over g) -> bf16 -> DRAM
"""

import os
import numpy as np

import concourse.bass as bass
import concourse.bacc as bacc
import concourse.mybir as mybir
from concourse.tile import TileContext
from contextlib import ExitStack

B, T, D, H = 4, 2048, 1024, 16
HD = D // H            # 64 head dim
P = 128                # partitions
KO = D // P            # 8 contraction subtiles for projections
TQT = 512              # tq tile width
NBLK = T // P          # 16 tk blocks
HPC = 8                # heads per core
G = 4                  # head-chunks per core (2 heads each)
JD = HPC * HD          # 512 local q/k/v dims
NCORES = 8
SCALE = 1.0 / float(np.sqrt(np.float32(HD)))

F32 = mybir.dt.float32
BF16 = mybir.dt.bfloat16

Exp = mybir.ActivationFunctionType.Exp
Copy = mybir.ActivationFunctionType.Copy
Mult = mybir.AluOpType.mult

# how 1/Z reaches the oT normalize: broadcast straight from PSUM (fast path)
# or staged via ACT copies into 32-aligned SBUF rows (fallback)
Z_MODE = os.environ.get("BASS_Z_MODE", "act_sbuf")
DEBUG = os.environ.get("BASS_DEBUG", "0") == "1"


def build_program():
    nc = bacc.Bacc("TRN2", target_bir_lowering=False, num_devices=NCORES)
    xt = nc.dram_tensor("xt", [D, T], BF16, kind="ExternalInput")
    wq = nc.dram_tensor("wq", [D, JD], BF16, kind="ExternalInput")
    wk = nc.dram_tensor("wk", [D, JD], BF16, kind="ExternalInput")
    wv = nc.dram_tensor("wv", [D, JD], BF16, kind="ExternalInput")
    wo = nc.dram_tensor("wo", [JD, D], BF16, kind="ExternalInput")
    cm = nc.dram_tensor("cmask", [P, P], F32, kind="ExternalInput")
    y = nc.dram_tensor("y", [T, D], BF16, kind="ExternalOutput")
    if DEBUG:
        dq = nc.dram_tensor("dq", [P, T], BF16, kind="ExternalOutput")
        dk = nc.dram_tensor("dk", [P, T], BF16, kind="ExternalOutput")
        dva = nc.dram_tensor("dva", [P, NBLK, HPC, HD + 1], BF16,
                             kind="ExternalOutput")
        det = nc.dram_tensor("det", [P, 2, TQT], BF16, kind="ExternalOutput")
        dz = nc.dram_tensor("dz", [33, TQT], F32, kind="ExternalOutput")
        do = nc.dram_tensor("do", [P, T], BF16, kind="ExternalOutput")
    dbg = {"dq": dq, "dk": dk, "dva": dva, "det": det, "dz": dz,
           "do": do} if DEBUG else None

    xt_r = xt[:].rearrange("(ko p) t -> p ko t", p=P)
    y_r = y[:].rearrange("(tn p) c -> p tn c", p=P)

    with TileContext(nc) as tc, ExitStack() as ctx:
        const = ctx.enter_context(tc.tile_pool(name="const", bufs=1))
        va_pool = ctx.enter_context(tc.tile_pool(name="va", bufs=1))
        qk_pool = ctx.enter_context(tc.tile_pool(name="qk", bufs=4))
        o_pool = ctx.enter_context(tc.tile_pool(name="o", bufs=4))
        e_pool = ctx.enter_context(tc.tile_pool(name="e", bufs=5))
        z_pool = ctx.enter_context(tc.tile_pool(name="z", bufs=4))
        y_pool = ctx.enter_context(tc.tile_pool(name="yp", bufs=4))
        psA = ctx.enter_context(tc.tile_pool(name="psA", bufs=2, space="PSUM"))
        psS = ctx.enter_context(tc.tile_pool(name="psS", bufs=2, space="PSUM"))
        psO = ctx.enter_context(tc.tile_pool(name="psO", bufs=2, space="PSUM"))

        # --- constants into SBUF ---
        x_sb = const.tile([P, KO, T], BF16, tag="x")

        def load_x(tt):
            nc.sync.dma_start(
                x_sb[:, :, tt * TQT : (tt + 1) * TQT],
                xt_r[:, :, tt * TQT : (tt + 1) * TQT],
            )

        load_x(0)
        wq_sb = const.tile([P, KO, JD], BF16, tag="wq")
        wk_sb = const.tile([P, KO, JD], BF16, tag="wk")
        wv_sb = const.tile([P, KO, JD], BF16, tag="wv")
        for w_sb, w_d in ((wq_sb, wq), (wk_sb, wk), (wv_sb, wv)):
            nc.sync.dma_start(w_sb, w_d[:].rearrange("(ko p) j -> p ko j", p=P))
        wo_sb = const.tile([P, G, D], BF16, tag="wo")
        # additive causal mask for the in-diagonal 128x128 triangle:
        # 0 where key<=query, -1e30 above the diagonal (same for every m)
        cm_sb = const.tile([P, 1, P], F32, tag="cm")
        nc.sync.dma_start(cm_sb[:, 0, :], cm[:])

        # v in natural layout + shared ones column per (block, head)
        va = va_pool.tile([P, NBLK, HPC, HD + 1], BF16, tag="va")
        nc.vector.tensor_copy(
            va[:, :, :, HD : HD + 1],
            nc.const_aps.tensor(1.0, (P, NBLK, HPC, 1), F32),
        )

        def stage_v(tn):
            """v projection for token block tn, all 8 heads, natural layout."""
            psv = psA.tile([P, HPC, HD], F32, tag="psA", name=f"psv{tn}")
            for ko in range(KO):
                nc.tensor.matmul(
                    psv,
                    x_sb[:, ko, tn * P : (tn + 1) * P],
                    wv_sb[:, ko, :],
                    start=(ko == 0),
                    stop=(ko == KO - 1),
                )
            nc.vector.tensor_copy(va[:, tn, :, 0:HD], psv)

        def stage_a_pair(g, qT, kT, pp):
            """q/k projections for head-chunk g, token tiles 2pp, 2pp+1.
            The two tiles share each stationary weight chunk."""
            js = slice(g * P, (g + 1) * P)
            tts = (2 * pp, 2 * pp + 1)
            for w_sb, dst in ((wq_sb, qT), (wk_sb, kT)):
                pps = [
                    psA.tile([P, TQT], F32, tag="psA", name=f"pp{s}")
                    for s in range(2)
                ]
                for ko in range(KO):
                    for s in range(2):
                        ts = slice(tts[s] * TQT, (tts[s] + 1) * TQT)
                        nc.tensor.matmul(
                            pps[s],
                            w_sb[:, ko, js],
                            x_sb[:, ko, ts],
                            start=(ko == 0),
                            stop=(ko == KO - 1),
                        )
                for s in range(2):
                    ts = slice(tts[s] * TQT, (tts[s] + 1) * TQT)
                    nc.vector.tensor_copy(dst[:, ts], pps[s])

        def stage_a_single(g, qT, kT, tt):
            """q/k projection for one token tile (fastest first-S ramp)."""
            js = slice(g * P, (g + 1) * P)
            ts = slice(tt * TQT, (tt + 1) * TQT)
            for w_sb, dst in ((wq_sb, qT), (wk_sb, kT)):
                pp = psA.tile([P, TQT], F32, tag="psA", name=f"ps{tt}")
                for ko in range(KO):
                    nc.tensor.matmul(
                        pp,
                        w_sb[:, ko, js],
                        x_sb[:, ko, ts],
                        start=(ko == 0),
                        stop=(ko == KO - 1),
                    )
                nc.vector.tensor_copy(dst[:, ts], pp)

        def stage_b_qt(g, qT, kT, oT, qt):
            """Attention for head-chunk g, query tile qt."""
            tq0 = qt * TQT
            nblk = qt * 4 + 4
            po = [
                psO.tile([HD + 1, TQT], F32, tag="po", name=f"po{h}")
                for h in range(2)
            ]
            for kb in range(nblk):
                m = kb - qt * 4  # >=0: diagonal-crossing block
                c0 = P * m if m >= 0 else 0
                ps2 = psS.tile([P, 2, TQT], F32, tag="ps")
                for h in range(2):
                    hs = slice(h * HD, (h + 1) * HD)
                    nc.tensor.matmul(
                        ps2[:, h, c0:TQT],
                        kT[hs, kb * P : (kb + 1) * P],
                        qT[hs, tq0 + c0 : tq0 + TQT],
                        start=True,
                        stop=True,
                    )
                et = e_pool.tile([P, 2, TQT], BF16, tag="et")
                nc.scalar.activation(
                    et[:, :, c0:TQT], ps2[:, :, c0:TQT], Exp, scale=SCALE
                )
                if m >= 0:
                    nc.vector.tensor_tensor(
                        et[:, :, c0 : c0 + P],
                        et[:, :, c0 : c0 + P],
                        cm_sb[:, 0:1, :].to_broadcast((P, 2, P)),
                        Mult,
                    )
                if DEBUG and g == 0 and qt == 0 and kb == 0:
                    nc.sync.dma_start(dbg["det"][:], et)
                for h in range(2):
                    nc.tensor.matmul(
                        po[h][:, c0:TQT],
                        va[:, kb, 2 * g + h, :],
                        et[:, h, c0:TQT],
                        start=(kb == 0),
                        stop=(kb == nblk - 1),
                    )
            # --- normalize: oT[hs, tq] = po[h][0:64] * (1/Z) ---
            # Z rows staged at physical partition 0: partition_broadcast
            # reads partition 0 of the source.
            for h in range(2):
                hs = slice(h * HD, (h + 1) * HD)
                zrow = z_pool.tile([1, TQT], F32, tag="zr", name=f"zr{h}")
                nc.vector.tensor_copy(zrow, po[h][HD : HD + 1, :])
                zrec = z_pool.tile([1, TQT], F32, tag="zc", name=f"zc{h}")
                nc.vector.reciprocal_approx_fast(zrec, zrow)
                rzb = z_pool.tile([P, TQT], F32, tag="rzb", name=f"rzb{h}")
                nc.gpsimd.partition_broadcast(rzb, zrec)
                nc.vector.tensor_tensor(
                    oT[hs, tq0 : tq0 + TQT],
                    po[h][0:HD, :],
                    rzb[hs, :],
                    Mult,
                )
                if DEBUG and g == 0 and qt == 0:
                    nc.sync.dma_start(dbg["dz"][:][32 * h : 32 * h + 1, :], zrow)

        def stage_c_part(oTs, part):
            """Quarter of the output projection (token blocks 4*part..)."""
            for tn in range(part * (NBLK // 4), (part + 1) * (NBLK // 4)):
                for cc in range(D // TQT):
                    psy = psA.tile([P, TQT], F32, tag="psA")
                    for g in range(G):
                        nc.tensor.matmul(
                            psy,
                            oTs[g][:, tn * P : (tn + 1) * P],
                            wo_sb[:, g, cc * TQT : (cc + 1) * TQT],
                            start=(g == 0),
                            stop=(g == G - 1),
                        )
                    yt = y_pool.tile([P, TQT], BF16, tag="yt")
                    nc.vector.tensor_copy(yt, psy)
                    nc.sync.dma_start(
                        y_r[:, tn, cc * TQT : (cc + 1) * TQT], yt
                    )

        # ---- schedule ----
        # qt-major sweeps: every sweep does vnat chunk, all 4 head-chunks'
        # JIT projection + attention for this query tile, then the previous
        # tile's output projection. Balances vnat/proj/C across the run.
        def new_qk(g):
            return (
                qk_pool.tile([P, T], BF16, tag="qT", name=f"qT{g}"),
                qk_pool.tile([P, T], BF16, tag="kT", name=f"kT{g}"),
            )

        qkT = {g: new_qk(g) for g in range(G)}
        oTs = {
            g: o_pool.tile([P, T], BF16, tag="oT", name=f"oT{g}")
            for g in range(G)
        }
        for qt in range(T // TQT):
            if qt + 1 < T // TQT:
                load_x(qt + 1)
            if qt == 1:
                nc.sync.dma_start(
                    wo_sb, wo[:].rearrange("(g p) i -> p g i", p=P)
                )
            for tn in range(4 * qt, 4 * qt + 4):
                stage_v(tn)
            for g in range(G):
                stage_a_single(g, *qkT[g], qt)
                stage_b_qt(g, *qkT[g], oTs[g], qt)
            if qt > 0:
                stage_c_part(oTs, qt - 1)
        stage_c_part(oTs, T // TQT - 1)
        if DEBUG:
            nc.sync.dma_start(dbg["dva"][:], va)
            nc.sync.dma_start(dbg["dq"][:], qkT[0][0])
            nc.sync.dma_start(dbg["dk"][:], qkT[0][1])
            nc.sync.dma_start(dbg["do"][:], oTs[0])

    nc.compile()
    return nc


def make_core_inputs(x, wq, wk, wv, wo):
    """Host-side sharding/layout prep. Returns list of 8 in_maps."""
    x = np.asarray(x, dtype=np.float32)
    wq = np.asarray(wq, dtype=np.float32)
    wk = np.asarray(wk, dtype=np.float32)
    wv = np.asarray(wv, dtype=np.float32)
    wo = np.asarray(wo, dtype=np.float32)
    mdt = mybir.dt.np(BF16)

    xts = [np.ascontiguousarray(x[b].T).astype(mdt) for b in range(B)]
    i = np.arange(P)[:, None]
    j = np.arange(P)[None, :]
    cmask = (i <= j).astype(np.float32)  # [P, P] multiplicative keep-mask

    in_maps = []
    for c in range(NCORES):
        b, hh = c // 2, c % 2
        js = slice(JD * hh, JD * (hh + 1))
        in_maps.append(
            {
                "xt": xts[b],
                "wq": np.ascontiguousarray(wq[js, :].T).astype(mdt),
                "wk": np.ascontiguousarray(wk[js, :].T).astype(mdt),
                "wv": np.ascontiguousarray(wv[js, :].T).astype(mdt),
                "wo": np.ascontiguousarray(wo[:, js].T).astype(mdt),
                "cmask": cmask,
            }
        )
    return in_maps


_CACHE = {}


def run(in_maps, **kwargs):
    from concourse.bass_utils import run_bass_kernel_spmd

    if "nc" not in _CACHE:
        _CACHE["nc"] = build_program()
    nc = _CACHE["nc"]
    res = run_bass_kernel_spmd(nc, in_maps, core_ids=list(range(NCORES)), **kwargs)
    return res


def kernel(x, wq, wk, wv, wo):
    in_maps = make_core_inputs(x, wq, wk, wv, wo)
    res = run(in_maps)
    y = np.zeros((B, T, D), dtype=np.float32)
    for c, r in enumerate(res.results):
        y[c // 2] += r["y"].astype(np.float32)
    return y
